# revision 49
# baseline (speedup 1.0000x reference)
"""Trainium2 Bass kernel for causal dynamic (MoE-routed) attention.

Problem: y = (softmax-routed top-4-of-16-heads causal attention)(x) @ W_o
  x [B=2, T=2048, D=1024], W_qkv [D, 3D], W_router [D, 16], W_o [D, D].

Sharding (8 cores): core c -> batch b = c // 4, head-group hg = c % 4
(4 of 16 heads). Each core computes a partial y contribution of its 4
heads for its batch; host sums the 4 partials per batch (row-parallel
W_o unshard) and stacks batches.

Routing exploit: the router (x @ W_router -> softmax -> top-4) is
computed on the HOST (tiny), so the device only runs attention for the
ACTIVE queries of each head.  Tokens are processed in windows of 256;
per (head, window) the active queries (mean 64, max 83 for the target
distribution) are compacted into NW=96 slots.

Device-side per core:
  - projections (f32r, full rate at >=256 free): K,V dim-/token-major,
    Q token-major, from xT staged in SBUF.
  - per (head h, window w): gather the active queries' Q columns via a
    0/1 gather matmul (P_g built on DVE from broadcast qidx vs iota),
    S = K^T Q_c [128k x 96q] per key block with causal masking applied
    by accumulating -1e30 * M1 into PSUM via an identity matmul (M1
    also built on DVE), exp on ACT (scale=1/8) -> PT bf16,
    PV in query-partition orientation: out[96q, 65] = PT^T @ [V | 1]
    (col 64 = softmax denominator), normalize on DVE, then scatter the
    gated head outputs back to token positions with a host-built
    scatter matrix (gates folded in) as a matmul into dim-major Y.
  - y_partial = Y @ W_o per 128-token block, staged and DMA'd out.
All attention-side matmuls are bf16 (1 cycle/row at any width).
"""

import os
import sys

import numpy as np

for _p in ("/opt/trn_rl_repo", "/root/.axon_site/_ro/trn_rl_repo"):
    if os.path.isdir(_p) and _p not in sys.path:
        sys.path.insert(0, _p)

import concourse.bacc as bacc
import concourse.bass as bass
import concourse.mybir as mybir
import concourse.tile as tile
from concourse.bass_utils import run_bass_kernel_spmd

F32 = mybir.dt.float32
F32R = mybir.dt.float32r
F16 = mybir.dt.float16
BF16 = mybir.dt.bfloat16
AF = mybir.ActivationFunctionType
ALU = mybir.AluOpType
AX = mybir.AxisListType

B = 2
D = 1024
H_TOTAL = 16
H_ACTIVE = 4
DH = 64          # head dim
HPC = 4          # heads per core
N_CORES = 8
WIN = 256        # token window
NEG_BIG = -1.0e30


def _bcast_inner(ap, n):
    """View a [P, 1] AP as [P, n] with step-0 innermost broadcast."""
    return bass.AP(
        tensor=ap.tensor,
        offset=ap.offset,
        ap=[*ap.ap[:-1], [0, n]],
    )


def _bcast_part(row_ap, parts):
    """View a [1, N] DRAM AP as [parts, N] via step-0 partition broadcast."""
    return bass.AP(
        tensor=row_ap.tensor,
        offset=row_ap.offset,
        ap=[[0, parts], row_ap.ap[-1]],
    )


def build_nc(T, NW):
    """Single-core Bass module (SPMD across 8 cores via inputs)."""
    NWIN = T // WIN       # 8 windows
    KB = T // 128         # 16 key blocks
    DC = D // 128         # 8 contraction chunks
    SGRP = 5              # S key-blocks per PSUM tile / exp call

    nc = bacc.Bacc("TRN2", target_bir_lowering=False, debug=False)

    xT = nc.dram_tensor("xT", [D, T], BF16, kind="ExternalInput")
    wk = nc.dram_tensor("wk", [D, 256], BF16, kind="ExternalInput")
    wq = nc.dram_tensor("wq", [D, 256], BF16, kind="ExternalInput")
    wv = nc.dram_tensor("wv", [D, 256], BF16, kind="ExternalInput")
    wo = nc.dram_tensor("wo", [256, D], BF16, kind="ExternalInput")
    pscat = nc.dram_tensor("pscat", [NW, NWIN * HPC * WIN], BF16,
                           kind="ExternalInput")
    qidxr = nc.dram_tensor("qidxr", [1, NWIN * HPC * NW], F16,
                           kind="ExternalInput")
    iotac = nc.dram_tensor("iotac", [128, KB], F32, kind="ExternalInput")
    xg = nc.dram_tensor("xg", [D, NWIN * HPC * NW], BF16,
                        kind="ExternalInput")
    idneg = nc.dram_tensor("idneg", [128, 128], BF16, kind="ExternalInput")
    out = nc.dram_tensor("out", [T, D], BF16, kind="ExternalOutput")

    with tile.TileContext(nc) as tc:
        with (
            tc.tile_pool(name="persist", bufs=1) as persist,
            tc.tile_pool(name="m1p", bufs=4) as m1p,
            tc.tile_pool(name="xgp", bufs=2) as xgp,
            tc.tile_pool(name="qcp", bufs=4) as qcp,
            tc.tile_pool(name="ptp", bufs=4) as ptp,
            tc.tile_pool(name="znp", bufs=8) as znp,
            tc.tile_pool(name="pvsp", bufs=8) as pvsp,
            tc.tile_pool(name="stgp", bufs=4) as stgp,
            tc.tile_pool(name="projps", bufs=2, space="PSUM") as projps,
            tc.tile_pool(name="sps", bufs=2, space="PSUM") as sps,
            tc.tile_pool(name="qps", bufs=1, space="PSUM") as qps,
            tc.tile_pool(name="pvps", bufs=2, space="PSUM") as pvps,
            tc.tile_pool(name="yps", bufs=1, space="PSUM") as yps,
        ):
            # ---- persistent SBUF ----
            # x^T staged as one tile, d-chunk major: col 2048*d + t
            xT_all = persist.tile([128, DC * T], BF16, tag="xTall",
                                  name="xTall")
            xT_sb = [xT_all[:, T * d:T * d + T] for d in range(DC)]
            wk_sb = persist.tile([128, DC * 256], BF16, tag="wk", name="wk")
            wq_sb = persist.tile([128, DC * 256], BF16, tag="wq", name="wq")
            wv_sb = persist.tile([128, DC * 256], BF16, tag="wv", name="wv")
            wo_sb = persist.tile([128, 2 * D], BF16, tag="wo", name="wo")
            KT = [persist.tile([128, T], BF16, tag=f"KT{t}", name=f"KT{t}")
                  for t in range(2)]
            Vt = persist.tile([128, KB * HPC * 65], BF16, tag="Vt", name="Vt")
            Vt4 = Vt.rearrange("p (k h e) -> p k h e", k=KB, h=HPC)
            Y = [persist.tile([128, T], BF16, tag=f"Y{t}", name=f"Y{t}")
                 for t in range(2)]
            # qidx broadcast to all partitions; col layout (w, h, c)
            qball = persist.tile([128, NWIN * HPC * NW], F16, tag="qball",
                                 name="qball")
            iota_sb = persist.tile([128, KB], F32, tag="iota", name="iota")
            idneg_sb = persist.tile([128, 128], BF16, tag="idneg",
                                    name="idneg")
            ps_sb = persist.tile([NW, NWIN * HPC * WIN], BF16, tag="ps",
                                 name="ps")
            zeroc = persist.tile([128, 1], F32, tag="zeroc", name="zeroc")

            # ---- input DMAs (tiny constants first, then what gates
            # the first projection matmuls) ----
            # PE warm-up: dependency-free matmuls on a memset tile keep
            # the clock-gate busy while bulk DMAs stream in.
            wrmsb = persist.tile([128, 128], BF16, tag="wrmsb", name="wrmsb")
            nc.gpsimd.memset(wrmsb, 0.0)
            wrm = projps.tile([128, 512], F32, tag="projps", name="warm")
            for i in range(48):
                nc.tensor.matmul(out=wrm[:, 0:128], lhsT=wrmsb,
                                 rhs=wrmsb, start=True, stop=True)

            nc.sync.dma_start(out=iota_sb, in_=iotac[:, :])
            nc.sync.dma_start(out=idneg_sb, in_=idneg[:, :])
            nc.gpsimd.memset(zeroc, 0.0)
            # ones columns of the augmented V (denominator trick)
            nc.gpsimd.memset(Vt4[:, :, :, 64:65], 1.0)

            def xquarter(q4):
                cs = slice(512 * q4, 512 * q4 + 512)
                nc.sync.dma_start(
                    out=xT_all.rearrange("p (d c) -> p d c", d=DC)[:, :, cs],
                    in_=xT[:, cs].rearrange("(d p) c -> p d c", p=128))

            xg_of = {}
            XGW = 2 * HPC * NW     # xg cols per w-pair per d-chunk

            def emit_xg(j):
                """DMA the gathered-x block for window pair (2j, 2j+1)."""
                xgt = xgp.tile([128, DC * XGW], BF16, tag="xg",
                               name=f"xg{j}")
                nc.sync.dma_start(
                    out=xgt.rearrange("p (d c) -> p d c", d=DC),
                    in_=xg[:, j * XGW:(j + 1) * XGW].rearrange(
                        "(d p) c -> p d c", p=128))
                xg_of[j] = xgt

            HQ = NWIN * HPC * NW // 2     # qball halves (w 0-3 | 4-7)
            HP = NWIN * HPC * WIN // 2    # pscat halves
            nc.sync.dma_start(
                out=wk_sb.rearrange("p (d c) -> p d c", d=DC),
                in_=wk[:, :].rearrange("(d p) c -> p d c", p=128))
            for dh in range(2):
                nc.sync.dma_start(
                    out=xT_all.rearrange("p (d c) -> p d c",
                                         d=DC)[:, 4 * dh:4 * dh + 4, 0:512],
                    in_=xT[512 * dh:512 * dh + 512, 0:512].rearrange(
                        "(d p) c -> p d c", p=128))
            nc.sync.dma_start(out=qball[:, 0:HQ],
                              in_=_bcast_part(qidxr[0:1, 0:HQ], 128))
            emit_xg(0)
            for wsb, wdr in ((wv_sb, wv), (wq_sb, wq)):
                nc.sync.dma_start(
                    out=wsb.rearrange("p (d c) -> p d c", d=DC),
                    in_=wdr[:, :].rearrange("(d p) c -> p d c", p=128))
            xquarter(1)
            nc.sync.dma_start(out=ps_sb[:, 0:HP], in_=pscat[:, 0:HP])
            nc.sync.dma_start(out=qball[:, HQ:],
                              in_=_bcast_part(qidxr[0:1, HQ:], 128))
            xquarter(2)
            nc.sync.dma_start(out=ps_sb[:, HP:], in_=pscat[:, HP:])
            nc.sync.dma_start(
                out=wo_sb.rearrange("p (t c) -> p t c", t=2),
                in_=wo[:, :].rearrange("(t p) c -> p t c", p=128))
            xquarter(3)

            zn_of = {}       # (h, w) -> zn tile
            pt_of = {}       # (h, w) -> PT tile

            def emit_pgm1(wlist):
                """DVE: build mask (M1) tiles for windows.

                One op covers all 4 heads (same iota scalar); tiles are
                [128, 2 chunks x 4 heads x NW], chunk-major.
                """
                for w in wlist:
                    m1 = m1p.tile([128, 2 * HPC * NW], BF16, tag="m1",
                                  name=f"m1{w}")
                    qsl = slice(w * HPC * NW, (w + 1) * HPC * NW)
                    for c in range(2):
                        kb = 2 * w + c
                        osl = slice(c * HPC * NW, (c + 1) * HPC * NW)
                        nc.vector.scalar_tensor_tensor(
                            out=m1[:, osl], in0=qball[:, qsl],
                            scalar=iota_sb[:, kb:kb + 1],
                            in1=_bcast_inner(zeroc, HPC * NW),
                            op0=ALU.subtract, op1=ALU.is_lt,
                        )
                    m1_of[w] = m1

            m1_of = {}

            def emit_proj(q4):
                """K, V, Q projections for token quarter q4 (512 tokens)."""
                cs = slice(q4 * 512, q4 * 512 + 512)
                for t in range(2):
                    ps = projps.tile([128, 512], F32, tag="projps",
                                     name=f"kproj{t}_{q4}")
                    for d in range(DC):
                        nc.tensor.matmul(
                            out=ps,
                            lhsT=wk_sb[:, 256 * d + 128 * t:
                                       256 * d + 128 * t + 128],
                            rhs=xT_sb[d][:, cs],
                            start=(d == 0), stop=(d == DC - 1),
                        )
                    nc.scalar.copy(out=KT[t][:, cs], in_=ps)
                for kb in range(4 * q4, 4 * q4 + 4):
                    tb = slice(128 * kb, 128 * kb + 128)
                    psv = projps.tile([128, 512], F32, tag="projps",
                                      name=f"vproj{kb}")
                    for d in range(DC):
                        nc.tensor.matmul(
                            out=psv[:, 0:256],
                            lhsT=xT_sb[d][:, tb],
                            rhs=wv_sb[:, 256 * d:256 * d + 256],
                            start=(d == 0), stop=(d == DC - 1),
                        )
                    nc.scalar.copy(
                        out=Vt4[:, kb, :, 0:64],
                        in_=psv[:, 0:256].rearrange("p (h e) -> p h e",
                                                    h=HPC),
                    )

            def emit_gather_s(wlist):
                """Q gather + S (+mask bias) + exp for the given windows."""
                for w in wlist:
                    nkb = 2 * w + 2
                    xgt = xg_of[w // 2]
                    qc_of = {}
                    for t in range(2):
                        # compact Q for both heads of the pair, directly
                        # from host-gathered x
                        psq = qps.tile([128, NW], F32, tag="qps",
                                       name=f"qg{t}_{w}")
                        for l in range(2):
                            h = 2 * t + l
                            xsl = ((w % 2) * HPC + h) * NW
                            for d in range(DC):
                                nc.tensor.matmul(
                                    out=psq[64 * l:64 * l + 64, :],
                                    lhsT=wq_sb[:, 256 * d + 64 * h:
                                               256 * d + 64 * h + 64],
                                    rhs=xgt[:, XGW * d + xsl:
                                            XGW * d + xsl + NW],
                                    start=(d == 0), stop=(d == DC - 1),
                                )
                        qc = qcp.tile([128, NW], BF16, tag="qc",
                                      name=f"qc{t}_{w}")
                        nc.vector.tensor_copy(out=qc, in_=psq)
                        qc_of[t] = qc
                    for t in range(2):
                        qc = qc_of[t]
                        for l in range(2):
                            h = 2 * t + l
                            m1 = m1_of[w]
                            pt = ptp.tile([128, KB * NW], BF16, tag="pt",
                                          name=f"pt{h}_{w}")
                            pt_of[(h, w)] = pt
                            for g0 in range(0, nkb, SGRP):
                                glen = min(SGRP, nkb - g0)
                                ps = sps.tile([128, SGRP * NW], F32, tag="sps",
                                              name=f"s{h}_{w}_{g0}")
                                for g in range(glen):
                                    kb = g0 + g
                                    osl = slice(g * NW, g * NW + NW)
                                    diag = kb >= 2 * w
                                    if diag:
                                        c = kb - 2 * w
                                        nc.tensor.matmul(
                                            out=ps[:, osl], lhsT=idneg_sb,
                                            rhs=m1[:, (c * HPC + h) * NW:
                                                   (c * HPC + h + 1) * NW],
                                            start=True, stop=False,
                                        )
                                    nc.tensor.matmul(
                                        out=ps[:, osl],
                                        lhsT=KT[t][64 * l:64 * l + 64,
                                                   128 * kb:128 * kb + 128],
                                        rhs=qc[64 * l:64 * l + 64, :],
                                        start=not diag, stop=True,
                                    )
                                nc.scalar.activation(
                                    out=pt[:, g0 * NW:(g0 + glen) * NW],
                                    in_=ps[:, 0:glen * NW],
                                    func=AF.Exp, scale=0.125,
                                )

            zn_of = {}

            def emit_pv(wlist):
                """PV + normalize (DVE straight from PSUM) for windows."""
                for w in wlist:
                    nkb = 2 * w + 2
                    for t in range(2):
                        for l in range(2):
                            h = 2 * t + l
                            pt = pt_of.pop((h, w))
                            psv = pvps.tile([NW, 65], F32, tag="pvps",
                                            name=f"pv{h}_{w}")
                            for kb in range(nkb):
                                nc.tensor.matmul(
                                    out=psv,
                                    lhsT=pt[:, kb * NW:kb * NW + NW],
                                    rhs=Vt4[:, kb, h, :],
                                    start=(kb == 0), stop=(kb == nkb - 1),
                                    skip_group_check=True,
                                )
                            # normalize straight out of PSUM on DVE
                            rcp = pvsp.tile([NW, 1], F32, tag="pvs",
                                            name=f"rcp{h}_{w}")
                            nc.vector.reciprocal(out=rcp, in_=psv[:, 64:65])
                            zn = znp.tile([NW, 64], BF16, tag="zn",
                                          name=f"zn{h}_{w}")
                            nc.vector.tensor_tensor(
                                out=zn, in0=psv[:, 0:64],
                                in1=_bcast_inner(rcp, 64),
                                op=ALU.mult,
                            )
                            zn_of[(h, w)] = zn

            def emit_scatter(wlist):
                """Gated scatter into dim-major Y for windows."""
                for w in wlist:
                    for t in range(2):
                        psy = yps.tile([128, WIN], F32, tag="yps",
                                       name=f"y{t}_{w}")
                        for l in range(2):
                            h = 2 * t + l
                            nc.tensor.matmul(
                                out=psy[64 * l:64 * l + 64, :],
                                lhsT=zn_of.pop((h, w)),
                                rhs=ps_sb[:, (w * HPC + h) * WIN:
                                          (w * HPC + h + 1) * WIN],
                                start=True, stop=True,
                                skip_group_check=True,
                            )
                        nc.vector.tensor_copy(
                            out=Y[t][:, WIN * w:WIN * w + WIN], in_=psy)

            def emit_pv_scatter(wlist):
                emit_pv(wlist)
                emit_scatter(wlist)

            def emit_wo(wlist, split_dma=False):
                """Output projection + DMA for the given windows' tokens."""
                for w in wlist:
                    for kb in (2 * w, 2 * w + 1):
                        tb = slice(128 * kb, 128 * kb + 128)
                        stage = stgp.tile([128, D], BF16, tag="stage",
                                          name=f"stage{kb}")
                        for nh in range(2):
                            nsl = slice(512 * nh, 512 * nh + 512)
                            ps = projps.tile([128, 512], F32, tag="projps",
                                             name=f"wops{kb}_{nh}")
                            for t in range(2):
                                nc.tensor.matmul(
                                    out=ps,
                                    lhsT=Y[t][:, tb],
                                    rhs=wo_sb[:, D * t + 512 * nh:
                                              D * t + 512 * nh + 512],
                                    start=(t == 0), stop=(t == 1),
                                )
                            if split_dma and nh == 1:
                                nc.scalar.copy(out=stage[:, nsl], in_=ps)
                            else:
                                nc.vector.tensor_copy(out=stage[:, nsl],
                                                      in_=ps)
                            if split_dma:
                                nc.sync.dma_start(out=out[tb, nsl],
                                                  in_=stage[:, nsl])
                        if not split_dma:
                            nc.sync.dma_start(out=out[tb, :], in_=stage)

            # ---------------- schedule ----------------
            emit_pgm1([0, 1])
            emit_proj(0)
            emit_xg(1)
            emit_gather_s([0, 1])
            emit_pgm1([2, 3])
            emit_proj(1)
            emit_pv_scatter([0])
            emit_gather_s([2])
            emit_pv_scatter([1])
            emit_gather_s([3])
            emit_xg(2)
            emit_pgm1([4, 5])
            emit_proj(2)
            emit_wo([0])
            emit_pv_scatter([2])
            emit_gather_s([4])
            emit_wo([1])
            emit_pv_scatter([3])
            emit_gather_s([5])
            emit_xg(3)
            emit_pgm1([6, 7])
            emit_proj(3)
            emit_wo([2])
            emit_pv_scatter([4])
            emit_gather_s([6])
            emit_wo([3])
            emit_pv_scatter([5])
            emit_gather_s([7])
            emit_wo([4])
            emit_pv([6])
            emit_wo([5])
            emit_scatter([6])
            emit_pv([7])
            emit_wo([6], split_dma=True)
            emit_scatter([7])
            emit_wo([7], split_dma=True)

    nc.compile()
    return nc


_NC_CACHE = {}
_LAST_NW = [88]


def _get_nc(T, NW=None):
    if NW is None:
        NW = _LAST_NW[0]
    key = (T, NW)
    if key not in _NC_CACHE:
        _NC_CACHE[key] = build_nc(T, NW)
    return _NC_CACHE[key]


def _softmax_f32(z):
    z = z - z.max(axis=-1, keepdims=True)
    e = np.exp(z, dtype=np.float32)
    return e / e.sum(axis=-1, keepdims=True)


def make_in_maps(x, W_qkv, W_router, W_o):
    """Host-side: router, compaction metadata, weight packing per core."""
    import ml_dtypes

    x = np.asarray(x, dtype=np.float32)
    W_qkv = np.asarray(W_qkv, dtype=np.float32)
    W_router = np.asarray(W_router, dtype=np.float32)
    W_o = np.asarray(W_o, dtype=np.float32)
    Bx, T, Dx = x.shape
    NWIN = T // WIN
    KB = T // 128

    # ---- router on host (f32, mirrors the reference) ----
    gates_all = []
    maxcnt = 0
    for b in range(Bx):
        probs = _softmax_f32(x[b] @ W_router)          # [T, 16]
        thresh = np.partition(probs, H_TOTAL - H_ACTIVE, axis=-1)[
            :, H_TOTAL - H_ACTIVE:H_TOTAL - H_ACTIVE + 1]
        gates = np.where(probs >= thresh, probs, 0.0).astype(np.float32)
        gates_all.append(gates)
        act = gates > 0
        cnt = act.reshape(NWIN, WIN, H_TOTAL).sum(1)
        maxcnt = max(maxcnt, int(cnt.max()))
    NW = max(88, -(-(maxcnt + 5) // 8) * 8)

    iotac = (np.arange(128, dtype=np.float32)[:, None]
             + 128.0 * np.arange(KB, dtype=np.float32)[None, :])
    iotac = np.ascontiguousarray(iotac)
    idneg = (NEG_BIG * np.eye(128, dtype=np.float32)).astype(
        ml_dtypes.bfloat16)

    in_maps = []
    for c in range(N_CORES):
        b, hg = c // 4, c % 4
        gates = gates_all[b]
        xT = np.ascontiguousarray(x[b].T).astype(ml_dtypes.bfloat16)
        wq = np.ascontiguousarray(
            W_qkv[:, 256 * hg:256 * hg + 256]).astype(ml_dtypes.bfloat16)
        wk = np.ascontiguousarray(
            W_qkv[:, 1024 + 256 * hg:1024 + 256 * hg + 256]).astype(
                ml_dtypes.bfloat16)
        wv = np.ascontiguousarray(
            W_qkv[:, 2048 + 256 * hg:2048 + 256 * hg + 256]).astype(
                ml_dtypes.bfloat16)
        wo = np.ascontiguousarray(
            W_o[256 * hg:256 * hg + 256, :]).astype(ml_dtypes.bfloat16)

        # qidxr col layout: (w, h, c) — matches qball slices on device
        qidxr = np.zeros((1, NWIN * HPC * NW), dtype=np.float16)
        pscat = np.zeros((NW, NWIN * HPC * WIN), dtype=np.float32)
        xgcols = np.zeros(NWIN * HPC * NW, dtype=np.int64)
        for hl in range(HPC):
            h = 4 * hg + hl
            for w in range(NWIN):
                idx = np.nonzero(gates[WIN * w:WIN * w + WIN, h])[0]
                n = len(idx)
                assert n <= NW, f"window overflow: {n} > {NW}"
                q0 = (w * HPC + hl) * NW
                qidxr[0, q0:q0 + n] = WIN * w + idx
                qidxr[0, q0 + n:q0 + NW] = WIN * w
                xgcols[q0:q0 + n] = WIN * w + idx
                xgcols[q0 + n:q0 + NW] = WIN * w
                col0 = (w * HPC + hl) * WIN
                pscat[np.arange(n), col0 + idx] = gates[WIN * w + idx, h]
        # gathered x columns for direct compact-Q projection
        xgarr = np.ascontiguousarray(xT[:, xgcols])
        in_maps.append({
            "xT": xT, "wk": wk, "wq": wq, "wv": wv, "wo": wo,
            "pscat": pscat.astype(ml_dtypes.bfloat16),
            "qidxr": qidxr, "iotac": iotac, "idneg": idneg, "xg": xgarr,
        })
    return in_maps, NW


def kernel_raw(x, W_qkv, W_router, W_o, **run_kwargs):
    """Run on the 8 cores; returns (full_output, BassKernelResults)."""
    import time

    T = x.shape[1]
    in_maps, NW = make_in_maps(x, W_qkv, W_router, W_o)
    _LAST_NW[0] = NW
    nc = _get_nc(T, NW)
    last_exc = None
    for attempt in range(3):
        try:
            res = run_bass_kernel_spmd(nc, in_maps,
                                       core_ids=list(range(N_CORES)),
                                       **run_kwargs)
            break
        except Exception as e:  # transient NRT_EXEC_UNIT_UNRECOVERABLE etc.
            last_exc = e
            if attempt == 2:
                raise
            time.sleep(20)
    partials = [np.asarray(r["out"], dtype=np.float32) for r in res.results]
    y = np.stack([
        partials[0] + partials[1] + partials[2] + partials[3],
        partials[4] + partials[5] + partials[6] + partials[7],
    ]).astype(np.float32)
    return y, res


def kernel(x, W_qkv, W_router, W_o):
    y, _ = kernel_raw(x, W_qkv, W_router, W_o)
    return y


# revision 50
# speedup vs baseline: 1.0819x; 1.0819x over previous
"""Trainium2 Bass kernel for causal dynamic (MoE-routed) attention.

Problem: y = (softmax-routed top-4-of-16-heads causal attention)(x) @ W_o
  x [B=2, T=2048, D=1024], W_qkv [D, 3D], W_router [D, 16], W_o [D, D].

Sharding (8 cores): core c -> batch b = c // 4, head-group hg = c % 4
(4 of 16 heads). Each core computes a partial y contribution of its 4
heads for its batch; host sums the 4 partials per batch (row-parallel
W_o unshard) and stacks batches.

Routing exploit: the router (x @ W_router -> softmax -> top-4) is
computed on the HOST (tiny), so the device only runs attention for the
ACTIVE queries of each head.  Tokens are processed in windows of 256;
per (head, window) the active queries (mean 64, max 83 for the target
distribution) are compacted into NW=96 slots.

Device-side per core:
  - projections (f32r, full rate at >=256 free): K,V dim-/token-major,
    Q token-major, from xT staged in SBUF.
  - per (head h, window w): gather the active queries' Q columns via a
    0/1 gather matmul (P_g built on DVE from broadcast qidx vs iota),
    S = K^T Q_c [128k x 96q] per key block with causal masking applied
    by accumulating -1e30 * M1 into PSUM via an identity matmul (M1
    also built on DVE), exp on ACT (scale=1/8) -> PT bf16,
    PV in query-partition orientation: out[96q, 65] = PT^T @ [V | 1]
    (col 64 = softmax denominator), normalize on DVE, then scatter the
    gated head outputs back to token positions with a host-built
    scatter matrix (gates folded in) as a matmul into dim-major Y.
  - y_partial = Y @ W_o per 128-token block, staged and DMA'd out.
All attention-side matmuls are bf16 (1 cycle/row at any width).
"""

import os
import sys

import numpy as np

for _p in ("/opt/trn_rl_repo", "/root/.axon_site/_ro/trn_rl_repo"):
    if os.path.isdir(_p) and _p not in sys.path:
        sys.path.insert(0, _p)

import concourse.bacc as bacc
import concourse.bass as bass
import concourse.mybir as mybir
import concourse.tile as tile
from concourse.bass_utils import run_bass_kernel_spmd

F32 = mybir.dt.float32
F32R = mybir.dt.float32r
F16 = mybir.dt.float16
BF16 = mybir.dt.bfloat16
AF = mybir.ActivationFunctionType
ALU = mybir.AluOpType
AX = mybir.AxisListType

B = 2
D = 1024
H_TOTAL = 16
H_ACTIVE = 4
DH = 64          # head dim
HPC = 4          # heads per core
N_CORES = 8
WIN = 256        # token window
NEG_BIG = -1.0e30


def _bcast_inner(ap, n):
    """View a [P, 1] AP as [P, n] with step-0 innermost broadcast."""
    return bass.AP(
        tensor=ap.tensor,
        offset=ap.offset,
        ap=[*ap.ap[:-1], [0, n]],
    )


def _bcast_part(row_ap, parts):
    """View a [1, N] DRAM AP as [parts, N] via step-0 partition broadcast."""
    return bass.AP(
        tensor=row_ap.tensor,
        offset=row_ap.offset,
        ap=[[0, parts], row_ap.ap[-1]],
    )


def build_nc(T, NW):
    """Single-core Bass module (SPMD across 8 cores via inputs)."""
    NWIN = T // WIN       # 8 windows
    KB = T // 128         # 16 key blocks
    DC = D // 128         # 8 contraction chunks
    SGRP = 5              # S key-blocks per PSUM tile / exp call

    nc = bacc.Bacc("TRN2", target_bir_lowering=False, debug=False)

    xT = nc.dram_tensor("xT", [D, T], BF16, kind="ExternalInput")
    wk = nc.dram_tensor("wk", [D, 256], BF16, kind="ExternalInput")
    wq = nc.dram_tensor("wq", [D, 256], BF16, kind="ExternalInput")
    wv = nc.dram_tensor("wv", [D, 256], BF16, kind="ExternalInput")
    wo = nc.dram_tensor("wo", [256, D], BF16, kind="ExternalInput")
    pscat = nc.dram_tensor("pscat", [NW, NWIN * HPC * WIN], BF16,
                           kind="ExternalInput")
    qidxr = nc.dram_tensor("qidxr", [1, NWIN * HPC * NW], F16,
                           kind="ExternalInput")
    iotac = nc.dram_tensor("iotac", [128, KB], F32, kind="ExternalInput")
    xg = nc.dram_tensor("xg", [D, NWIN * HPC * NW], BF16,
                        kind="ExternalInput")
    idneg = nc.dram_tensor("idneg", [128, 128], BF16, kind="ExternalInput")
    out = nc.dram_tensor("out", [T, D], BF16, kind="ExternalOutput")

    with tile.TileContext(nc) as tc:
        with (
            tc.tile_pool(name="persist", bufs=1) as persist,
            tc.tile_pool(name="m1p", bufs=4) as m1p,
            tc.tile_pool(name="xgp", bufs=2) as xgp,
            tc.tile_pool(name="qcp", bufs=4) as qcp,
            tc.tile_pool(name="ptp", bufs=4) as ptp,
            tc.tile_pool(name="znp", bufs=8) as znp,
            tc.tile_pool(name="pvsp", bufs=8) as pvsp,
            tc.tile_pool(name="stgp", bufs=4) as stgp,
            tc.tile_pool(name="projps", bufs=2, space="PSUM") as projps,
            tc.tile_pool(name="sps", bufs=2, space="PSUM") as sps,
            tc.tile_pool(name="qps", bufs=1, space="PSUM") as qps,
            tc.tile_pool(name="pvps", bufs=2, space="PSUM") as pvps,
            tc.tile_pool(name="yps", bufs=1, space="PSUM") as yps,
        ):
            # ---- persistent SBUF ----
            # x^T staged as one tile, d-chunk major: col 2048*d + t
            xT_all = persist.tile([128, DC * T], BF16, tag="xTall",
                                  name="xTall")
            xT_sb = [xT_all[:, T * d:T * d + T] for d in range(DC)]
            wk_sb = persist.tile([128, DC * 256], BF16, tag="wk", name="wk")
            wq_sb = persist.tile([128, DC * 256], BF16, tag="wq", name="wq")
            wv_sb = persist.tile([128, DC * 256], BF16, tag="wv", name="wv")
            wo_sb = persist.tile([128, 2 * D], BF16, tag="wo", name="wo")
            KT = [persist.tile([128, T], BF16, tag=f"KT{t}", name=f"KT{t}")
                  for t in range(2)]
            Vt = persist.tile([128, KB * HPC * 65], BF16, tag="Vt", name="Vt")
            Vt4 = Vt.rearrange("p (k h e) -> p k h e", k=KB, h=HPC)
            Y = [persist.tile([128, T], BF16, tag=f"Y{t}", name=f"Y{t}")
                 for t in range(2)]
            # qidx broadcast to all partitions; col layout (w, h, c)
            qball = persist.tile([128, NWIN * HPC * NW], F16, tag="qball",
                                 name="qball")
            iota_sb = persist.tile([128, KB], F32, tag="iota", name="iota")
            idneg_sb = persist.tile([128, 128], BF16, tag="idneg",
                                    name="idneg")
            ps_sb = persist.tile([NW, NWIN * HPC * WIN], BF16, tag="ps",
                                 name="ps")
            zeroc = persist.tile([128, 1], F32, tag="zeroc", name="zeroc")

            # ---- input DMAs (tiny constants first, then what gates
            # the first projection matmuls) ----
            # PE warm-up: dependency-free matmuls on a memset tile keep
            # the clock-gate busy while bulk DMAs stream in.
            wrmsb = persist.tile([128, 128], BF16, tag="wrmsb", name="wrmsb")
            nc.gpsimd.memset(wrmsb, 0.0)
            wrm = projps.tile([128, 512], F32, tag="projps", name="warm")
            for i in range(48):
                nc.tensor.matmul(out=wrm[:, 0:128], lhsT=wrmsb,
                                 rhs=wrmsb, start=True, stop=True)

            nc.sync.dma_start(out=iota_sb, in_=iotac[:, :])
            nc.sync.dma_start(out=idneg_sb, in_=idneg[:, :])
            nc.gpsimd.memset(zeroc, 0.0)
            # ones columns of the augmented V (denominator trick)
            nc.gpsimd.memset(Vt4[:, :, :, 64:65], 1.0)

            def xquarter(q4):
                cs = slice(512 * q4, 512 * q4 + 512)
                nc.sync.dma_start(
                    out=xT_all.rearrange("p (d c) -> p d c", d=DC)[:, :, cs],
                    in_=xT[:, cs].rearrange("(d p) c -> p d c", p=128))

            xg_of = {}
            XGW = 2 * HPC * NW     # xg cols per w-pair per d-chunk

            def emit_xg(j):
                """DMA the gathered-x block for window pair (2j, 2j+1)."""
                xgt = xgp.tile([128, DC * XGW], BF16, tag="xg",
                               name=f"xg{j}")
                nc.sync.dma_start(
                    out=xgt.rearrange("p (d c) -> p d c", d=DC),
                    in_=xg[:, j * XGW:(j + 1) * XGW].rearrange(
                        "(d p) c -> p d c", p=128))
                xg_of[j] = xgt

            HQ = NWIN * HPC * NW // 2     # qball halves (w 0-3 | 4-7)
            HP = NWIN * HPC * WIN // 2    # pscat halves
            nc.sync.dma_start(
                out=wk_sb.rearrange("p (d c) -> p d c", d=DC),
                in_=wk[:, :].rearrange("(d p) c -> p d c", p=128))
            for dh in range(2):
                nc.sync.dma_start(
                    out=xT_all.rearrange("p (d c) -> p d c",
                                         d=DC)[:, 4 * dh:4 * dh + 4, 0:512],
                    in_=xT[512 * dh:512 * dh + 512, 0:512].rearrange(
                        "(d p) c -> p d c", p=128))
            nc.sync.dma_start(
                out=wv_sb.rearrange("p (d c) -> p d c", d=DC),
                in_=wv[:, :].rearrange("(d p) c -> p d c", p=128))
            xquarter(1)
            nc.sync.dma_start(out=qball[:, 0:HQ],
                              in_=_bcast_part(qidxr[0:1, 0:HQ], 128))
            emit_xg(0)
            nc.sync.dma_start(
                out=wq_sb.rearrange("p (d c) -> p d c", d=DC),
                in_=wq[:, :].rearrange("(d p) c -> p d c", p=128))
            nc.sync.dma_start(out=ps_sb[:, 0:HP], in_=pscat[:, 0:HP])
            nc.sync.dma_start(out=qball[:, HQ:],
                              in_=_bcast_part(qidxr[0:1, HQ:], 128))
            emit_xg(1)
            xquarter(2)
            nc.sync.dma_start(out=ps_sb[:, HP:], in_=pscat[:, HP:])
            nc.sync.dma_start(
                out=wo_sb.rearrange("p (t c) -> p t c", t=2),
                in_=wo[:, :].rearrange("(t p) c -> p t c", p=128))
            xquarter(3)

            zn_of = {}       # (h, w) -> zn tile
            pt_of = {}       # (h, w) -> PT tile

            def emit_pgm1(wlist):
                """DVE: build mask (M1) tiles for windows.

                One op covers all 4 heads (same iota scalar); tiles are
                [128, 2 chunks x 4 heads x NW], chunk-major.
                """
                for w in wlist:
                    m1 = m1p.tile([128, 2 * HPC * NW], BF16, tag="m1",
                                  name=f"m1{w}")
                    qsl = slice(w * HPC * NW, (w + 1) * HPC * NW)
                    for c in range(2):
                        kb = 2 * w + c
                        osl = slice(c * HPC * NW, (c + 1) * HPC * NW)
                        nc.vector.scalar_tensor_tensor(
                            out=m1[:, osl], in0=qball[:, qsl],
                            scalar=iota_sb[:, kb:kb + 1],
                            in1=_bcast_inner(zeroc, HPC * NW),
                            op0=ALU.subtract, op1=ALU.is_lt,
                        )
                    m1_of[w] = m1

            m1_of = {}

            def emit_proj(q4):
                """K, V, Q projections for token quarter q4 (512 tokens)."""
                cs = slice(q4 * 512, q4 * 512 + 512)
                for t in range(2):
                    ps = projps.tile([128, 512], F32, tag="projps",
                                     name=f"kproj{t}_{q4}")
                    for d in range(DC):
                        nc.tensor.matmul(
                            out=ps,
                            lhsT=wk_sb[:, 256 * d + 128 * t:
                                       256 * d + 128 * t + 128],
                            rhs=xT_sb[d][:, cs],
                            start=(d == 0), stop=(d == DC - 1),
                        )
                    nc.scalar.copy(out=KT[t][:, cs], in_=ps)
                for kb in range(4 * q4, 4 * q4 + 4):
                    tb = slice(128 * kb, 128 * kb + 128)
                    psv = projps.tile([128, 512], F32, tag="projps",
                                      name=f"vproj{kb}")
                    for d in range(DC):
                        nc.tensor.matmul(
                            out=psv[:, 0:256],
                            lhsT=xT_sb[d][:, tb],
                            rhs=wv_sb[:, 256 * d:256 * d + 256],
                            start=(d == 0), stop=(d == DC - 1),
                        )
                    nc.scalar.copy(
                        out=Vt4[:, kb, :, 0:64],
                        in_=psv[:, 0:256].rearrange("p (h e) -> p h e",
                                                    h=HPC),
                    )

            def emit_gather_s(wlist):
                """Q gather + S (+mask bias) + exp for the given windows."""
                for w in wlist:
                    nkb = 2 * w + 2
                    xgt = xg_of[w // 2]
                    qc_of = {}
                    for t in range(2):
                        # compact Q for both heads of the pair, directly
                        # from host-gathered x
                        psq = qps.tile([128, NW], F32, tag="qps",
                                       name=f"qg{t}_{w}")
                        for l in range(2):
                            h = 2 * t + l
                            xsl = ((w % 2) * HPC + h) * NW
                            for d in range(DC):
                                nc.tensor.matmul(
                                    out=psq[64 * l:64 * l + 64, :],
                                    lhsT=wq_sb[:, 256 * d + 64 * h:
                                               256 * d + 64 * h + 64],
                                    rhs=xgt[:, XGW * d + xsl:
                                            XGW * d + xsl + NW],
                                    start=(d == 0), stop=(d == DC - 1),
                                )
                        qc = qcp.tile([128, NW], BF16, tag="qc",
                                      name=f"qc{t}_{w}")
                        nc.vector.tensor_copy(out=qc, in_=psq)
                        qc_of[t] = qc
                    for t in range(2):
                        qc = qc_of[t]
                        for l in range(2):
                            h = 2 * t + l
                            m1 = m1_of[w]
                            pt = ptp.tile([128, KB * NW], BF16, tag="pt",
                                          name=f"pt{h}_{w}")
                            pt_of[(h, w)] = pt
                            for g0 in range(0, nkb, SGRP):
                                glen = min(SGRP, nkb - g0)
                                ps = sps.tile([128, SGRP * NW], F32, tag="sps",
                                              name=f"s{h}_{w}_{g0}")
                                for g in range(glen):
                                    kb = g0 + g
                                    osl = slice(g * NW, g * NW + NW)
                                    diag = kb >= 2 * w
                                    if diag:
                                        c = kb - 2 * w
                                        nc.tensor.matmul(
                                            out=ps[:, osl], lhsT=idneg_sb,
                                            rhs=m1[:, (c * HPC + h) * NW:
                                                   (c * HPC + h + 1) * NW],
                                            start=True, stop=False,
                                        )
                                    nc.tensor.matmul(
                                        out=ps[:, osl],
                                        lhsT=KT[t][64 * l:64 * l + 64,
                                                   128 * kb:128 * kb + 128],
                                        rhs=qc[64 * l:64 * l + 64, :],
                                        start=not diag, stop=True,
                                    )
                                nc.scalar.activation(
                                    out=pt[:, g0 * NW:(g0 + glen) * NW],
                                    in_=ps[:, 0:glen * NW],
                                    func=AF.Exp, scale=0.125,
                                )

            zn_of = {}

            def emit_pv(wlist):
                """PV + normalize (DVE straight from PSUM) for windows."""
                for w in wlist:
                    nkb = 2 * w + 2
                    for t in range(2):
                        for l in range(2):
                            h = 2 * t + l
                            pt = pt_of.pop((h, w))
                            psv = pvps.tile([NW, 65], F32, tag="pvps",
                                            name=f"pv{h}_{w}")
                            for kb in range(nkb):
                                nc.tensor.matmul(
                                    out=psv,
                                    lhsT=pt[:, kb * NW:kb * NW + NW],
                                    rhs=Vt4[:, kb, h, :],
                                    start=(kb == 0), stop=(kb == nkb - 1),
                                    skip_group_check=True,
                                )
                            # normalize straight out of PSUM on DVE
                            rcp = pvsp.tile([NW, 1], F32, tag="pvs",
                                            name=f"rcp{h}_{w}")
                            nc.vector.reciprocal(out=rcp, in_=psv[:, 64:65])
                            zn = znp.tile([NW, 64], BF16, tag="zn",
                                          name=f"zn{h}_{w}")
                            nc.vector.tensor_tensor(
                                out=zn, in0=psv[:, 0:64],
                                in1=_bcast_inner(rcp, 64),
                                op=ALU.mult,
                            )
                            zn_of[(h, w)] = zn

            def emit_scatter(wlist):
                """Gated scatter into dim-major Y for windows."""
                for w in wlist:
                    for t in range(2):
                        psy = yps.tile([128, WIN], F32, tag="yps",
                                       name=f"y{t}_{w}")
                        for l in range(2):
                            h = 2 * t + l
                            nc.tensor.matmul(
                                out=psy[64 * l:64 * l + 64, :],
                                lhsT=zn_of.pop((h, w)),
                                rhs=ps_sb[:, (w * HPC + h) * WIN:
                                          (w * HPC + h + 1) * WIN],
                                start=True, stop=True,
                                skip_group_check=True,
                            )
                        nc.vector.tensor_copy(
                            out=Y[t][:, WIN * w:WIN * w + WIN], in_=psy)

            def emit_pv_scatter(wlist):
                emit_pv(wlist)
                emit_scatter(wlist)

            def emit_wo(wlist, split_dma=False):
                """Output projection + DMA for the given windows' tokens."""
                for w in wlist:
                    for kb in (2 * w, 2 * w + 1):
                        tb = slice(128 * kb, 128 * kb + 128)
                        stage = stgp.tile([128, D], BF16, tag="stage",
                                          name=f"stage{kb}")
                        for nh in range(2):
                            nsl = slice(512 * nh, 512 * nh + 512)
                            ps = projps.tile([128, 512], F32, tag="projps",
                                             name=f"wops{kb}_{nh}")
                            for t in range(2):
                                nc.tensor.matmul(
                                    out=ps,
                                    lhsT=Y[t][:, tb],
                                    rhs=wo_sb[:, D * t + 512 * nh:
                                              D * t + 512 * nh + 512],
                                    start=(t == 0), stop=(t == 1),
                                )
                            if split_dma and nh == 1:
                                nc.scalar.copy(out=stage[:, nsl], in_=ps)
                            else:
                                nc.vector.tensor_copy(out=stage[:, nsl],
                                                      in_=ps)
                            if split_dma:
                                nc.sync.dma_start(out=out[tb, nsl],
                                                  in_=stage[:, nsl])
                        if not split_dma:
                            nc.sync.dma_start(out=out[tb, :], in_=stage)

            # ---------------- schedule ----------------
            emit_pgm1([0, 1])
            emit_proj(0)
            emit_proj(1)
            emit_gather_s([0, 1])
            emit_xg(2)
            emit_pgm1([2, 3])
            emit_proj(2)
            emit_pv_scatter([0])
            emit_gather_s([2])
            emit_pv_scatter([1])
            emit_gather_s([3])
            emit_xg(3)
            emit_pgm1([4, 5])
            emit_proj(3)
            emit_wo([0])
            emit_pv_scatter([2])
            emit_gather_s([4])
            emit_wo([1])
            emit_pv_scatter([3])
            emit_gather_s([5])
            emit_pgm1([6, 7])
            emit_wo([2])
            emit_pv_scatter([4])
            emit_gather_s([6])
            emit_wo([3])
            emit_pv_scatter([5])
            emit_gather_s([7])
            emit_wo([4])
            emit_pv([6])
            emit_wo([5])
            emit_scatter([6])
            emit_pv([7])
            emit_wo([6], split_dma=True)
            emit_scatter([7])
            emit_wo([7], split_dma=True)

    nc.compile()
    return nc


_NC_CACHE = {}
_LAST_NW = [88]


def _get_nc(T, NW=None):
    if NW is None:
        NW = _LAST_NW[0]
    key = (T, NW)
    if key not in _NC_CACHE:
        _NC_CACHE[key] = build_nc(T, NW)
    return _NC_CACHE[key]


def _softmax_f32(z):
    z = z - z.max(axis=-1, keepdims=True)
    e = np.exp(z, dtype=np.float32)
    return e / e.sum(axis=-1, keepdims=True)


def make_in_maps(x, W_qkv, W_router, W_o):
    """Host-side: router, compaction metadata, weight packing per core."""
    import ml_dtypes

    x = np.asarray(x, dtype=np.float32)
    W_qkv = np.asarray(W_qkv, dtype=np.float32)
    W_router = np.asarray(W_router, dtype=np.float32)
    W_o = np.asarray(W_o, dtype=np.float32)
    Bx, T, Dx = x.shape
    NWIN = T // WIN
    KB = T // 128

    # ---- router on host (f32, mirrors the reference) ----
    gates_all = []
    maxcnt = 0
    for b in range(Bx):
        probs = _softmax_f32(x[b] @ W_router)          # [T, 16]
        thresh = np.partition(probs, H_TOTAL - H_ACTIVE, axis=-1)[
            :, H_TOTAL - H_ACTIVE:H_TOTAL - H_ACTIVE + 1]
        gates = np.where(probs >= thresh, probs, 0.0).astype(np.float32)
        gates_all.append(gates)
        act = gates > 0
        cnt = act.reshape(NWIN, WIN, H_TOTAL).sum(1)
        maxcnt = max(maxcnt, int(cnt.max()))
    NW = max(88, -(-(maxcnt + 5) // 8) * 8)

    iotac = (np.arange(128, dtype=np.float32)[:, None]
             + 128.0 * np.arange(KB, dtype=np.float32)[None, :])
    iotac = np.ascontiguousarray(iotac)
    idneg = (NEG_BIG * np.eye(128, dtype=np.float32)).astype(
        ml_dtypes.bfloat16)

    in_maps = []
    for c in range(N_CORES):
        b, hg = c // 4, c % 4
        gates = gates_all[b]
        xT = np.ascontiguousarray(x[b].T).astype(ml_dtypes.bfloat16)
        wq = np.ascontiguousarray(
            W_qkv[:, 256 * hg:256 * hg + 256]).astype(ml_dtypes.bfloat16)
        wk = np.ascontiguousarray(
            W_qkv[:, 1024 + 256 * hg:1024 + 256 * hg + 256]).astype(
                ml_dtypes.bfloat16)
        wv = np.ascontiguousarray(
            W_qkv[:, 2048 + 256 * hg:2048 + 256 * hg + 256]).astype(
                ml_dtypes.bfloat16)
        wo = np.ascontiguousarray(
            W_o[256 * hg:256 * hg + 256, :]).astype(ml_dtypes.bfloat16)

        # qidxr col layout: (w, h, c) — matches qball slices on device
        qidxr = np.zeros((1, NWIN * HPC * NW), dtype=np.float16)
        pscat = np.zeros((NW, NWIN * HPC * WIN), dtype=np.float32)
        xgcols = np.zeros(NWIN * HPC * NW, dtype=np.int64)
        for hl in range(HPC):
            h = 4 * hg + hl
            for w in range(NWIN):
                idx = np.nonzero(gates[WIN * w:WIN * w + WIN, h])[0]
                n = len(idx)
                assert n <= NW, f"window overflow: {n} > {NW}"
                q0 = (w * HPC + hl) * NW
                qidxr[0, q0:q0 + n] = WIN * w + idx
                qidxr[0, q0 + n:q0 + NW] = WIN * w
                xgcols[q0:q0 + n] = WIN * w + idx
                xgcols[q0 + n:q0 + NW] = WIN * w
                col0 = (w * HPC + hl) * WIN
                pscat[np.arange(n), col0 + idx] = gates[WIN * w + idx, h]
        # gathered x columns for direct compact-Q projection
        xgarr = np.ascontiguousarray(xT[:, xgcols])
        in_maps.append({
            "xT": xT, "wk": wk, "wq": wq, "wv": wv, "wo": wo,
            "pscat": pscat.astype(ml_dtypes.bfloat16),
            "qidxr": qidxr, "iotac": iotac, "idneg": idneg, "xg": xgarr,
        })
    return in_maps, NW


def kernel_raw(x, W_qkv, W_router, W_o, **run_kwargs):
    """Run on the 8 cores; returns (full_output, BassKernelResults)."""
    import time

    T = x.shape[1]
    in_maps, NW = make_in_maps(x, W_qkv, W_router, W_o)
    _LAST_NW[0] = NW
    nc = _get_nc(T, NW)
    last_exc = None
    for attempt in range(3):
        try:
            res = run_bass_kernel_spmd(nc, in_maps,
                                       core_ids=list(range(N_CORES)),
                                       **run_kwargs)
            break
        except Exception as e:  # transient NRT_EXEC_UNIT_UNRECOVERABLE etc.
            last_exc = e
            if attempt == 2:
                raise
            time.sleep(20)
    partials = [np.asarray(r["out"], dtype=np.float32) for r in res.results]
    y = np.stack([
        partials[0] + partials[1] + partials[2] + partials[3],
        partials[4] + partials[5] + partials[6] + partials[7],
    ]).astype(np.float32)
    return y, res


def kernel(x, W_qkv, W_router, W_o):
    y, _ = kernel_raw(x, W_qkv, W_router, W_o)
    return y
